# revision 62
# baseline (speedup 1.0000x reference)
"""Bass/Tile TRN2 kernel for nn_DiagonalLSTM (v4).

Data-parallel over batch: 16 batch elements across 8 cores -> 2 per core.
Per core, two independent pipelines ("groups", one per batch element)
run the 128-step LSTM scan.  The serial per-step dependency chain - not
engine throughput - bounds the runtime, so the design minimizes it:

  sh(t-1) -> taps-mms -> sigmoid(gates) -> [cgs4, t1, u4, add] DVE block
          -> tanh(c) -> sh(t)

- PSUM is initialized by a dependency-free zero matmul, and the x-term
  matmuls (valid rows only, stride-127 diagonal reads of natural-layout
  bf16 x) depend only on the input DMA: both run ahead; only the two
  bf16 state-tap matmuls sit on the chain (real NTFF trace: 260ns
  cold-weights + 108ns warm, LDWEIGHTS ~100ns pipelined alongside).
- The prev-tap (row shift by one) is the same ring slot read one column
  earlier; column 0 of each slot is a permanent zero pad.
- All matmul operands live at partition base 0: members of one PSUM
  accumulation group must share a PE row band (mixed tile_positions
  crash the device).
- One act table for everything: the sigmoid_and_others set holds both
  Sigmoid and Tanh; a dummy preamble activation pulls the ~1.3us table
  load into the input-DMA wait.  c-state kept as CH4 = 2c; candidate
  gate via cgs4 = 4*sigmoid(2g)-2 = 2*tanh(g) with one dual-scalar
  tensor_scalar; the 2x inside sigmoid(2g) is pre-scaled into the gg
  weight columns on the host (pre2) so sigma needs no per-partition
  scale operand (measured ~85ns/step on HW); the ring stores full
  h = tch * sig_o.
- Engine placement (real-NTFF-measured; the chain is ENGINE-EXEC
  bound: DVE ~214ns/op, Sigmoid 367/Tanh 400, sem gaps only 10-90ns
  -- the CoreSim cost model has this backwards): the WHOLE
  elementwise cell runs on DVE - per group a
  queue-contiguous block [cgs4, t1, u4, add] (cgs first, t1 second
  fills the cgs->u4 RAW-drain bubble) and the final h-multiply.  Pool
  (gpsimd) is ~430ns/op loaded vs DVE ~375 and lost every HW A/B race.
  The cgs realign is forced by the verifier's equal-base rule for
  2-input DVE ops (cross-quadrant operands reject or corrupt).  bf16
  cell tiles enable the DVE 2x perf mode.
- Startup: x DMAs issue lo-row-halves first (hi halves are first read
  at t=64, ~190us in); SP issues serially at ~650ns each so issue
  order gates the scan start.
- Dead ends measured on HW, kept out: Pool for t1/h (slower per op);
  qsplit (direct-Tanh gg: +1 ACT op serializes what the -1 DVE op
  saves); merged two-group tanh (lockstep coupling costs ~80us);
  affine_mul_reduce custom-DVE cgs+u4 fusion (cross-base operands
  corrupt on HW); packed weight/bias tiles as SBUF operands (sliced /
  strided engine operands cost ~600ns/step; packed DRAM with sliced
  DMA reads into dedicated tiles is fine).
- The whole cell is bf16 except biases; measured rel err 9.7e-3 on HW
  (gate 2e-2, deterministic inputs).
- Output is a raw ring dump every CHUNK=8 steps (contiguous SBUF DMA),
  decoded on the host.
- timing=True builds declare all data tensors Internal so repeated
  calls move no host bytes (used by test.py's interleaved differencing).
"""

import sys

sys.path.insert(0, "/opt/trn_rl_repo")

from contextlib import ExitStack

import numpy as np

import concourse.bass as bass
import concourse.tile as tile
from concourse import bacc, mybir

F32 = mybir.dt.float32
BF16 = mybir.dt.bfloat16
AF = mybir.ActivationFunctionType
ALU = mybir.AluOpType

N_CORES = 8
B = 2  # batch per core (= groups)
CIN = 32
H = 128  # rows
T = 128  # scan steps
BO = 32
G4 = 4 * BO  # gate channels, partition order (o, f, i, gg)
RS = 130  # ring slot columns: [pad, 128 rows, spare]
R = 16  # ring depth (slots)
CHUNK = 8  # output DMA chunk (steps); divides T, <= R/2
LOOSE_U = False  # the HW birverifier (NCC_IBIR297) demands equal base
# partitions for 2-input SBUF DVE ops, so u needs the cgs realign first
CELLDT = BF16  # cell-state dtype: bf16 enables the DVE 2x perf mode
MERGED = False  # merged-tanh variant (group-B gates reordered)
PRE2 = True  # gg 2x pre-scaled into weights: sigma needs no scale operand


def _build_module(reps=1, t_steps=None, no_out=False, timing=False,
                  no_x=False, no_tail=False, no_cell=False,
                  pool_add=False, t1_dve=True, h_dve=True,
                  qsplit=False, bufs=2, psum_bufs=2, merged=False,
                  pre2=True, t1_first=False, t1_split=0):
    TS = T if t_steps is None else t_steps
    nc = bacc.Bacc(
        "TRN2",
        target_bir_lowering=False,
        debug=False,
        num_devices=N_CORES,
    )

    # timing builds take no external data (uninitialized internal DRAM;
    # instruction stream and therefore timing are identical) so repeated
    # calls move no host bytes
    ki = "Internal" if timing else "ExternalInput"
    ko = "Internal" if timing else "ExternalOutput"
    # natural-layout x, bf16, all at partitions 0-31 (cin), 4 column
    # blocks of 64*T: (g0 rows 0-63, g0 rows 64-127, g1 lo, g1 hi).
    # Matmuls may only mix within one PE row band: every matmul operand
    # lives at partition base 0 (mixed tile_positions in one PSUM
    # accumulation group crash the device).
    # merged: per-group gate orders -> per-group weight/bias copies
    NW = B if merged else 1
    xc_d = nc.dram_tensor("xc", [CIN, 4 * 64 * T], BF16, kind=ki)
    wpack_d = nc.dram_tensor("wpack", [BO, NW * 2 * G4], BF16, kind=ki)
    w2t_d = nc.dram_tensor("w2t", [CIN, NW * G4], BF16, kind=ki)
    bsc_d = nc.dram_tensor("bsc", [G4, NW * 2], F32, kind=ki)
    # raw ring dump: [group, chunk, 32 gates, CHUNK slots * RS cols]
    hs_d = nc.dram_tensor(
        "hs", [B, TS // CHUNK, BO, CHUNK * RS], BF16, kind=ko
    )
    tiny_d = (
        nc.dram_tensor("tiny", [1, 4], F32, kind="ExternalOutput")
        if timing else None
    )

    with ExitStack() as ctx:
        tc = ctx.enter_context(tile.TileContext(nc))
        const = ctx.enter_context(tc.tile_pool(name="const", bufs=1))
        psum = ctx.enter_context(
            tc.tile_pool(name="psum", bufs=psum_bufs, space="PSUM")
        )
        sig_p = ctx.enter_context(tc.tile_pool(name="sig", bufs=bufs))
        tmp_p = ctx.enter_context(tc.tile_pool(name="tmp", bufs=bufs))

        # ---- persistent tiles ----
        xc = const.tile([CIN, 4 * 64 * T], BF16, tag="xc")
        zq = const.tile([BO, H], BF16, tag="zq")  # zero matmul rhs
        zl1 = const.tile([1, G4], BF16, tag="zl1")  # contraction-1 zero lhsT
        # ring: h(r) at slot col 1+r; group g at col offset g * R * RS
        ring = const.tile([BO, B * R * RS], BF16, tag="ring")
        # per-group weight/bias tiles (shared when not merged)
        wtap, wprev0, w2t4, biast, scalet = {}, {}, {}, {}, {}
        for gw in range(NW):
            wtap[gw] = const.tile([BO, G4], BF16, tag=f"wtap{gw}",
                                  name=f"wtap{gw}")
            wprev0[gw] = const.tile([BO, G4], BF16, tag=f"wprev0{gw}",
                                    name=f"wprev0{gw}")
            w2t4[gw] = const.tile([CIN, G4], BF16, tag=f"w2t4{gw}",
                                  name=f"w2t4{gw}")
            biast[gw] = const.tile([G4, 1], F32, tag=f"biast{gw}",
                                   name=f"biast{gw}")
            scalet[gw] = const.tile([G4, 1], F32, tag=f"scalet{gw}",
                                    name=f"scalet{gw}")
        for g in range(B):
            gw = g % NW
            wtap[g], wprev0[g], w2t4[g] = wtap[gw], wprev0[gw], w2t4[gw]
            biast[g], scalet[g] = biast[gw], scalet[gw]
        zb = const.tile([G4, 1], F32, tag="zb")
        dummy = const.tile([BO, 4], CELLDT, tag="dummy")
        # gate-quadrant bases per group.  Non-merged: both groups use
        # (o,f,i,gg).  Merged: A=(o,i,f,gg), B=(i,o,gg,f) so the two
        # groups' cell rows land adjacent at partitions 64:128 (APs may
        # span 64 partitions only from base 0 or 64).
        if merged:
            O0 = {0: 0, 1: BO}
            I0 = {0: BO, 1: 0}
            F0 = {0: 2 * BO, 1: 3 * BO}
            GG0 = {0: 3 * BO, 1: 2 * BO}
            chAB = const.tile([G4, H], CELLDT, tag="chAB")
            chs = {g: chAB[F0[g] : F0[g] + BO, :] for g in range(B)}
        else:
            O0 = {0: 0, 1: 0}
            F0 = {0: BO, 1: BO}
            I0 = {0: 2 * BO, 1: 2 * BO}
            GG0 = {0: 3 * BO, 1: 3 * BO}
            ch = {g: const.tile([2 * BO, H], CELLDT, tag=f"ch{g}",
                                name=f"ch{g}")
                  for g in range(B)}
            chs = {g: ch[g][BO : 2 * BO, :] for g in range(B)}

        # ---- preamble ----
        nc.vector.memset(zb[:, :], 0.0)
        nc.vector.memset(zl1[:, :], 0.0)
        nc.vector.memset(zq[:, :], 0.0)
        # dummy activation: pulls the act-table load (~1.3us) into the
        # DMA wait instead of the first real sigmoid
        nc.scalar.activation(
            dummy[:, :], zq[:, 0:4], AF.Sigmoid, bias=zb[0:BO, :], scale=1.0
        )
        for gw in range(NW):
            o = gw * 2 * G4
            nc.sync.dma_start(
                out=wtap[gw][:, :], in_=wpack_d.ap()[:, o : o + G4]
            )
            nc.sync.dma_start(
                out=wprev0[gw][:, :],
                in_=wpack_d.ap()[:, o + G4 : o + 2 * G4],
            )
            nc.sync.dma_start(
                out=w2t4[gw][:, :],
                in_=w2t_d.ap()[:, gw * G4 : (gw + 1) * G4],
            )
            nc.sync.dma_start(
                out=biast[gw][:, :], in_=bsc_d.ap()[:, 2 * gw : 2 * gw + 1]
            )
            nc.sync.dma_start(
                out=scalet[gw][:, :],
                in_=bsc_d.ap()[:, 2 * gw + 1 : 2 * gw + 2],
            )
            if not qsplit:
                # legacy sigmoid(2z) trick wants the gg bias doubled
                gg0 = GG0[gw] if merged else 3 * BO
                nc.vector.tensor_scalar(
                    biast[gw][gg0 : gg0 + BO, :],
                    biast[gw][gg0 : gg0 + BO, :],
                    2.0, None, ALU.mult,
                )
        # x in 16-row chunks: lo halves (rows 0-63) first -- the hi halves
        # are first read at t=64, ~190us into the scan (the SP sequencer
        # issues DMAs serially at ~650ns each; issue order gates startup)
        for q in range(4):
            for blk in (0, 2):
                c0 = blk * 64 * T + q * 16 * T
                nc.sync.dma_start(
                    out=xc[:, c0 : c0 + 16 * T],
                    in_=xc_d.ap()[:, c0 : c0 + 16 * T],
                )
        for q in range(4):
            for blk in (1, 3):
                c0 = blk * 64 * T + q * 16 * T
                nc.sync.dma_start(
                    out=xc[:, c0 : c0 + 16 * T],
                    in_=xc_d.ap()[:, c0 : c0 + 16 * T],
                )

        # slot R-1 (read by step 0's taps) first, then the rest
        for g in range(B):
            o = g * R * RS
            nc.vector.memset(ring[:, o + (R - 1) * RS : o + R * RS], 0.0)
        for g in range(B):
            o = g * R * RS
            nc.vector.memset(ring[:, o : o + (R - 1) * RS], 0.0)
        if merged:
            nc.vector.memset(chAB[:, :], 0.0)
        else:
            for g in range(B):
                nc.vector.memset(ch[g][:, :], 0.0)

        rv = ring[:, :].rearrange("p (g s c) -> p g s c", g=B, s=R)
        xv = xc[:, :].rearrange("p (b c) -> p b c", b=4)  # 4 column blocks

        # ---- the scan ----
        import contextlib

        rep_ctx = tc.For_i(0, reps, 1) if reps > 1 else contextlib.nullcontext()
        with rep_ctx:
            for t in range(TS):
                sp = (t - 1) % R
                sl = t % R

                def mm_phase(g):
                    gp = psum.tile([G4, H], F32, tag=f"g{g}", name=f"g{g}")
                    # PSUM init: contraction-1 zero matmul (real-HW MM
                    # time scales with contraction via weight load; a
                    # 32-row zero-mm cost 260ns of PE occupancy that
                    # could collide with the chain-critical taps)
                    nc.tensor.matmul(
                        gp[:, :], zl1[:, :], zq[0:1, :],
                        start=True, stop=False,
                    )
                    # x-term: valid rows only (diagonal stride-127 reads);
                    # off the serial chain (depends only on the input DMA)
                    if not no_x:
                        nlo = min(t + 1, 64)
                        nc.tensor.matmul(
                            gp[:, 0:nlo], w2t4[g][:, :],
                            xv[:, 2 * g, t : t + 127 * (nlo - 1) + 1 : 127],
                            start=False, stop=False,
                        )
                        if t >= 64:
                            nhi = t - 64 + 1
                            d0 = t - 64
                            nc.tensor.matmul(
                                gp[:, 64 : 64 + nhi], w2t4[g][:, :],
                                xv[:, 2 * g + 1,
                                   d0 : d0 + 127 * (nhi - 1) + 1 : 127],
                                start=False, stop=False,
                            )
                    # state taps (the only mms on the serial chain).
                    # The h-wait rides the lowered Ldweights, so a tap
                    # that waits for h starts cold (~150ns weight fill
                    # on the chain).  Warm-up trick: a 1-col matmul on
                    # the ring's PAD column (col 0: constant zero,
                    # written only by the preamble memset -> NO per-step
                    # deps) executes during the h-wait, pre-loading wtap
                    # and accumulating exactly 0; the real tap then
                    # starts warm (~108ns).
                    nc.tensor.matmul(
                        gp[:, 0:1], wtap[g][:, :],
                        rv[0:BO, g, sp, 0:1],
                        start=False, stop=False,
                    )
                    nc.tensor.matmul(
                        gp[:, :], wtap[g][:, :],
                        rv[0:BO, g, sp, 1 : 1 + H],
                        start=False, stop=False,
                    )
                    nc.tensor.matmul(
                        gp[:, :], wprev0[g][:, :],
                        rv[0:BO, g, sp, 0:H],
                        start=False, stop=True,
                    )
                    return gp

                gps = [mm_phase(g) for g in range(B)]

                sg, w, t1, tg = {}, {}, {}, {}
                tchs = {}
                for g in range(B):
                    sg[g] = sig_p.tile([G4, H], CELLDT, tag=f"sg{g}",
                                       name=f"sg{g}")
                    if qsplit:
                        # sigmoid on o,f,i quadrants only (scale folded to
                        # 1.0: no scale operand fetch)
                        nc.scalar.activation(
                            sg[g][0 : 3 * BO, :], gps[g][0 : 3 * BO, :],
                            AF.Sigmoid, bias=biast[g][0 : 3 * BO, :],
                        )
                        # gg quadrant: direct tanh, realigned to the
                        # i-quadrant base so the u4 stt sees equal bases
                        tg[g] = tmp_p.tile(
                            [G4, H], CELLDT, tag=f"tg{g}",
                            name=f"tg{g}")[I0[g] : I0[g] + BO, :]
                        nc.scalar.activation(
                            tg[g], gps[g][3 * BO : 4 * BO, :],
                            AF.Tanh, bias=biast[g][3 * BO : 4 * BO, :],
                        )
                    else:
                        nc.scalar.activation(
                            sg[g][:, :], gps[g][:, :], AF.Sigmoid,
                            bias=biast[g][:, :],
                            scale=1.0 if pre2 else scalet[g][:, :],
                        )
                if no_cell:
                    for g in range(B):
                        # timing ablation: ring write straight from sg
                        nc.vector.scalar_tensor_tensor(
                            rv[0:BO, g, sl, 1 : 1 + H],
                            sg[g][O0[g] : O0[g] + BO, :],
                            0.5, sg[g][O0[g] : O0[g] + BO, :],
                            ALU.subtract, ALU.mult,
                        )
                    continue
                # t1 on Pool: all groups when not t1_dve, or only the
                # groups selected by the t1_split bitmask (Pool is idle;
                # one Pool t1 avoids the both-on-Pool queueing loss)
                t1_on_pool = {g: (not t1_dve) or bool(t1_split & (1 << g))
                              for g in range(B)}
                for g in range(B):
                    if t1_on_pool[g]:
                        t1[g] = tmp_p.tile(
                            [G4, H], CELLDT, tag=f"t1{g}",
                            name=f"t1{g}")[F0[g] : F0[g] + BO, :]
                        nc.gpsimd.tensor_tensor(
                            t1[g], sg[g][F0[g] : F0[g] + BO, :],
                            chs[g], ALU.mult,
                        )
                # DVE trio per group, group-major so the other group's ops
                # don't interleave into this group's chain
                for g in range(B):
                    if g not in w:
                        w[g] = tmp_p.tile([G4, H], CELLDT, tag=f"w{g}",
                                          name=f"w{g}")
                    if qsplit:
                        if t1_dve:
                            # t1 = sig_f * CH4
                            t1[g] = tmp_p.tile(
                                [G4, H], CELLDT, tag=f"t1{g}",
                                name=f"t1{g}")[F0[g] : F0[g] + BO, :]
                            nc.vector.tensor_tensor(
                                t1[g], sg[g][F0[g] : F0[g] + BO, :],
                                chs[g], ALU.mult,
                            )
                        # u4 = (tanh_gg * 2) * sig_i  (equal bases at I0)
                        nc.vector.scalar_tensor_tensor(
                            w[g][F0[g] : F0[g] + BO, :], tg[g], 2.0,
                            sg[g][I0[g] : I0[g] + BO, :],
                            ALU.mult, ALU.mult,
                        )
                        eng_add = nc.gpsimd if pool_add else nc.vector
                        eng_add.tensor_tensor(
                            chs[g], w[g][F0[g] : F0[g] + BO, :],
                            t1[g], ALU.add,
                        )
                        continue
                    def _cgs():
                        # cgs4 = 4*sigmoid(2g_gg) - 2 = 2*tanh(g_gg),
                        # realigned from the gg quadrant to the i base
                        nc.vector.tensor_scalar(
                            w[g][I0[g] : I0[g] + BO, :],
                            sg[g][GG0[g] : GG0[g] + BO, :],
                            4.0, 2.0, ALU.mult, ALU.subtract,
                        )

                    def _t1():
                        # t1 = sig_f * CH4 (fills the cgs->u4 RAW-drain
                        # bubble when second)
                        t1[g] = tmp_p.tile(
                            [G4, H], CELLDT, tag=f"t1{g}",
                            name=f"t1{g}")[F0[g] : F0[g] + BO, :]
                        nc.vector.tensor_tensor(
                            t1[g], sg[g][F0[g] : F0[g] + BO, :],
                            chs[g], ALU.mult,
                        )

                    if t1_first and not t1_on_pool[g]:
                        _t1(); _cgs()
                    else:
                        _cgs()
                        if not t1_on_pool[g]:
                            _t1()
                    # u4 = cgs4 * sig_i = 2*i*gg -> the f-quadrant base
                    nc.vector.tensor_tensor(
                        w[g][F0[g] : F0[g] + BO, :],
                        w[g][I0[g] : I0[g] + BO, :],
                        sg[g][I0[g] : I0[g] + BO, :], ALU.mult,
                    )
                    # CH4 = u4 + t1 (queue-contiguous: no sem hop on chain)
                    eng_add = nc.gpsimd if pool_add else nc.vector
                    eng_add.tensor_tensor(
                        chs[g], w[g][F0[g] : F0[g] + BO, :],
                        t1[g], ALU.add,
                    )
                if no_tail:
                    for g in range(B):
                        # timing ablation: ring write from CH4, no sigma4
                        nc.vector.scalar_tensor_tensor(
                            rv[0:BO, g, sl, 1 : 1 + H],
                            chs[g], 0.5, t1[g],
                            ALU.subtract, ALU.mult,
                        )
                    continue
                # tch = tanh(0.5*CH4) = tanh(c) (same act table as
                # Sigmoid: sigmoid_and_others has both -> no reload)
                if merged:
                    # both groups' cell rows are adjacent (64:128): ONE
                    # tanh op + one sem instead of two
                    tchAB = tmp_p.tile([2 * BO, H], CELLDT, tag="tchAB",
                                       name="tchAB")
                    nc.scalar.activation(
                        tchAB[:, :], chAB[2 * BO : 4 * BO, :], AF.Tanh,
                        bias=zb[2 * BO : 4 * BO, :], scale=0.5,
                    )
                    for g in range(B):
                        tchs[g] = tchAB[F0[g] - 2 * BO :
                                        F0[g] - 2 * BO + BO, :]
                else:
                    for g in range(B):
                        tch_t = tmp_p.tile([BO, H], CELLDT, tag=f"tch{g}",
                                           name=f"tch{g}")
                        nc.scalar.activation(
                            tch_t[:, :], chs[g], AF.Tanh,
                            bias=zb[BO : 2 * BO, :], scale=0.5,
                        )
                        tchs[g] = tch_t[:, :]
                for g in range(B):
                    # sh = tch * sig_o = o*tanh(c) = h -> ring hA
                    eng_h = nc.vector if h_dve else nc.gpsimd
                    eng_h.tensor_tensor(
                        rv[0:BO, g, sl, 1 : 1 + H], tchs[g],
                        sg[g][O0[g] : O0[g] + BO, :], ALU.mult,
                    )

                if t % CHUNK == CHUNK - 1 and not no_out:
                    # contiguous SBUF source (1 descriptor per partition)
                    c0 = t - CHUNK + 1
                    s0 = c0 % R
                    for g in range(B):
                        o0 = (g * R + s0) * RS
                        nc.sync.dma_start(
                            out=hs_d.ap()[g, t // CHUNK, :, :],
                            in_=ring[0:BO, o0 : o0 + CHUNK * RS],
                        )

        if timing:
            # tiny real output so the bass_exec call cannot be elided
            tt = const.tile([1, 4], F32, tag="tt")
            nc.vector.tensor_copy(tt[:, :], ring[0:1, 0:4])
            nc.sync.dma_start(out=tiny_d.ap(), in_=tt[:, :])

    nc.compile()
    return nc


_NC_CACHE = {}


def _get_module(**kw):
    key = tuple(sorted(kw.items()))
    if key not in _NC_CACHE:
        _NC_CACHE[key] = _build_module(**kw)
    return _NC_CACHE[key]


def _prep_weights(W2, b2, W1, b1, merged=False, pre2=False):
    import ml_dtypes

    W2 = np.asarray(W2, np.float32)
    W1 = np.asarray(W1, np.float32)
    b = np.asarray(b1, np.float32) + np.asarray(b2, np.float32)
    bias = b.copy()  # plain; legacy sigmoid(2z) path doubles gg on device
    scale = np.ones(G4, np.float32)
    scale[3 * BO :] = 2.0
    bf = ml_dtypes.bfloat16
    wcur = np.ascontiguousarray(W1[:, :, 1].T).astype(bf)
    wprev = np.ascontiguousarray(W1[:, :, 0].T).astype(bf)
    w2t = np.ascontiguousarray(W2.T).astype(bf)
    # gate orders (source layout is (o,f,i,gg)):
    #   non-merged: identity for both groups
    #   merged:     A=(o,i,f,gg), B=(i,o,gg,f)
    o_b, f_b, i_b, g_b = (np.arange(k * BO, (k + 1) * BO) for k in range(4))
    if merged:
        perms = [
            np.concatenate([o_b, i_b, f_b, g_b]),
            np.concatenate([i_b, o_b, g_b, f_b]),
        ]
    else:
        perms = [np.arange(G4)]
    if pre2:
        # pre-double the gg columns so sigma needs no scale operand
        # (the sigmoid(2z) trick's 2x rides in the weights)
        for m in (wcur, wprev, w2t):
            m[:, 3 * BO :] = (m[:, 3 * BO :].astype(np.float32) * 2.0
                              ).astype(m.dtype)
    wps, w2s, bscs = [], [], []
    for p in perms:
        wps += [wcur[:, p], wprev[:, p]]
        w2s.append(w2t[:, p])
        bscs += [bias[p], scale[p]]
    wpack = np.ascontiguousarray(np.concatenate(wps, axis=1))
    w2tp = np.ascontiguousarray(np.concatenate(w2s, axis=1))
    bsc = np.ascontiguousarray(np.stack(bscs, axis=1))
    return wpack, w2tp, bsc


def _prep_canvas(x):
    """Per-core natural-layout x [CIN, 4*64*T] bf16, partitions 0-31,
    column blocks (g0 rows 0-63, g0 rows 64-127, g1 lo, g1 hi)."""
    import ml_dtypes

    nb, _, _, _ = x.shape  # (16, CIN, H, T)
    out = np.empty((nb // B, CIN, 4 * 64 * T), np.float32)
    for k in range(nb // B):
        for g in range(B):
            xb = x[B * k + g]  # (CIN, H, T)
            o = 2 * g * 64 * T
            out[k, :, o : o + 64 * T] = xb[:, 0:64, :].reshape(CIN, -1)
            out[k, :, o + 64 * T : o + 2 * 64 * T] = (
                xb[:, 64:128, :].reshape(CIN, -1)
            )
    return out.astype(ml_dtypes.bfloat16)


def kernel(x, W2, b2, W1, b1):
    from concourse.bass_utils import run_bass_kernel_spmd

    nc = _get_module(merged=MERGED, pre2=PRE2)
    x = np.ascontiguousarray(x, dtype=np.float32)
    wpack, w2t, bsc = _prep_weights(W2, b2, W1, b1, merged=MERGED,
                                    pre2=PRE2)
    xcs = _prep_canvas(x)
    in_maps = [
        {"xc": xcs[k], "wpack": wpack, "w2t": w2t, "bsc": bsc}
        for k in range(N_CORES)
    ]
    res = run_bass_kernel_spmd(nc, in_maps, list(range(N_CORES)))
    out = np.empty((N_CORES * B, BO, H, T), np.float32)
    for k in range(N_CORES):
        hs = _decode_hs(res.results[k]["hs"])
        out[2 * k : 2 * k + 2] = hs
    return out


def _decode_hs(hs):
    """(B, T//CHUNK, BO, CHUNK*RS) raw ring dump -> (B, BO, H, T) = 2*SH."""
    hs = np.asarray(hs, np.float32).reshape(B, T // CHUNK, BO, CHUNK, RS)
    hs = hs[:, :, :, :, 1 : 1 + H]  # (B, nch, BO, CHUNK, H=row)
    hs = hs.transpose(0, 2, 4, 1, 3).reshape(B, BO, H, T)
    return hs



# revision 64
# speedup vs baseline: 1.0061x; 1.0061x over previous
"""Bass/Tile TRN2 kernel for nn_DiagonalLSTM (v4).

Data-parallel over batch: 16 batch elements across 8 cores -> 2 per core.
Per core, two independent pipelines ("groups", one per batch element)
run the 128-step LSTM scan.  The serial per-step dependency chain - not
engine throughput - bounds the runtime, so the design minimizes it:

  sh(t-1) -> taps-mms -> sigmoid(gates) -> [cgs4, t1, u4, add] DVE block
          -> tanh(c) -> sh(t)

- PSUM is initialized by a dependency-free zero matmul, and the x-term
  matmuls (valid rows only, stride-127 diagonal reads of natural-layout
  bf16 x) depend only on the input DMA: both run ahead; only the two
  bf16 state-tap matmuls sit on the chain (real NTFF trace: 260ns
  cold-weights + 108ns warm, LDWEIGHTS ~100ns pipelined alongside).
- The prev-tap (row shift by one) is the same ring slot read one column
  earlier; column 0 of each slot is a permanent zero pad.
- All matmul operands live at partition base 0: members of one PSUM
  accumulation group must share a PE row band (mixed tile_positions
  crash the device).
- One act table for everything: the sigmoid_and_others set holds both
  Sigmoid and Tanh; a dummy preamble activation pulls the ~1.3us table
  load into the input-DMA wait.  c-state kept as CH4 = 2c; candidate
  gate via cgs4 = 4*sigmoid(2g)-2 = 2*tanh(g) with one dual-scalar
  tensor_scalar; the 2x inside sigmoid(2g) is pre-scaled into the gg
  weight columns on the host (pre2) so sigma needs no per-partition
  scale operand (measured ~85ns/step on HW); the ring stores full
  h = tch * sig_o.
- Engine placement (real-NTFF-measured; the chain is ENGINE-EXEC
  bound: DVE ~214ns/op, Sigmoid 367/Tanh 400, sem gaps only 10-90ns
  -- the CoreSim cost model has this backwards): the WHOLE
  elementwise cell runs on DVE - per group a
  queue-contiguous block [cgs4, t1, u4, add] (cgs first, t1 second
  fills the cgs->u4 RAW-drain bubble) and the final h-multiply.  Pool
  (gpsimd) is ~430ns/op loaded vs DVE ~375 and lost every HW A/B race.
  The cgs realign is forced by the verifier's equal-base rule for
  2-input DVE ops (cross-quadrant operands reject or corrupt).  bf16
  cell tiles enable the DVE 2x perf mode.
- Startup: x DMAs issue lo-row-halves first (hi halves are first read
  at t=64, ~190us in); SP issues serially at ~650ns each so issue
  order gates the scan start.
- Dead ends measured on HW, kept out: Pool for t1/h (slower per op);
  qsplit (direct-Tanh gg: +1 ACT op serializes what the -1 DVE op
  saves); merged two-group tanh (lockstep coupling costs ~80us);
  affine_mul_reduce custom-DVE cgs+u4 fusion (cross-base operands
  corrupt on HW); packed weight/bias tiles as SBUF operands (sliced /
  strided engine operands cost ~600ns/step; packed DRAM with sliced
  DMA reads into dedicated tiles is fine).
- The whole cell is bf16 except biases; measured rel err 9.7e-3 on HW
  (gate 2e-2, deterministic inputs).
- Output is a raw ring dump every CHUNK=8 steps (contiguous SBUF DMA),
  decoded on the host.
- timing=True builds declare all data tensors Internal so repeated
  calls move no host bytes (used by test.py's interleaved differencing).
"""

import sys

sys.path.insert(0, "/opt/trn_rl_repo")

from contextlib import ExitStack

import numpy as np

import concourse.bass as bass
import concourse.tile as tile
from concourse import bacc, mybir

F32 = mybir.dt.float32
BF16 = mybir.dt.bfloat16
AF = mybir.ActivationFunctionType
ALU = mybir.AluOpType

N_CORES = 8
B = 2  # batch per core (= groups)
CIN = 32
H = 128  # rows
T = 128  # scan steps
BO = 32
G4 = 4 * BO  # gate channels, partition order (o, f, i, gg)
RS = 130  # ring slot columns: [pad, 128 rows, spare]
R = 16  # ring depth (slots)
CHUNK = 8  # output DMA chunk (steps); divides T, <= R/2
LOOSE_U = False  # the HW birverifier (NCC_IBIR297) demands equal base
# partitions for 2-input SBUF DVE ops, so u needs the cgs realign first
CELLDT = BF16  # cell-state dtype: bf16 enables the DVE 2x perf mode
MERGED = False  # merged-tanh variant (group-B gates reordered)
PRE2 = True  # gg 2x pre-scaled into weights: sigma needs no scale operand


def _build_module(reps=1, t_steps=None, no_out=False, timing=False,
                  no_x=False, no_tail=False, no_cell=False,
                  pool_add=False, t1_dve=True, h_dve=True,
                  qsplit=False, bufs=2, psum_bufs=2, merged=False,
                  pre2=True, t1_first=False, t1_split=0):
    TS = T if t_steps is None else t_steps
    nc = bacc.Bacc(
        "TRN2",
        target_bir_lowering=False,
        debug=False,
        num_devices=N_CORES,
    )

    # timing builds take no external data (uninitialized internal DRAM;
    # instruction stream and therefore timing are identical) so repeated
    # calls move no host bytes
    ki = "Internal" if timing else "ExternalInput"
    ko = "Internal" if timing else "ExternalOutput"
    # natural-layout x, bf16, all at partitions 0-31 (cin), 4 column
    # blocks of 64*T: (g0 rows 0-63, g0 rows 64-127, g1 lo, g1 hi).
    # Matmuls may only mix within one PE row band: every matmul operand
    # lives at partition base 0 (mixed tile_positions in one PSUM
    # accumulation group crash the device).
    # merged: per-group gate orders -> per-group weight/bias copies
    NW = B if merged else 1
    xc_d = nc.dram_tensor("xc", [CIN, 4 * 64 * T], BF16, kind=ki)
    wpack_d = nc.dram_tensor("wpack", [BO, NW * 2 * G4], BF16, kind=ki)
    w2t_d = nc.dram_tensor("w2t", [CIN, NW * G4], BF16, kind=ki)
    bsc_d = nc.dram_tensor("bsc", [G4, NW * 2], F32, kind=ki)
    # raw ring dump: [group, chunk, 32 gates, CHUNK slots * RS cols]
    hs_d = nc.dram_tensor(
        "hs", [B, TS // CHUNK, BO, CHUNK * RS], BF16, kind=ko
    )
    tiny_d = (
        nc.dram_tensor("tiny", [1, 4], F32, kind="ExternalOutput")
        if timing else None
    )

    with ExitStack() as ctx:
        tc = ctx.enter_context(tile.TileContext(nc))
        const = ctx.enter_context(tc.tile_pool(name="const", bufs=1))
        psum = ctx.enter_context(
            tc.tile_pool(name="psum", bufs=psum_bufs, space="PSUM")
        )
        sig_p = ctx.enter_context(tc.tile_pool(name="sig", bufs=bufs))
        tmp_p = ctx.enter_context(tc.tile_pool(name="tmp", bufs=bufs))

        # ---- persistent tiles ----
        xc = const.tile([CIN, 4 * 64 * T], BF16, tag="xc")
        zq = const.tile([BO, H], BF16, tag="zq")  # zero matmul rhs
        zl1 = const.tile([1, G4], BF16, tag="zl1")  # contraction-1 zero lhsT
        # ring: h(r) at slot col 1+r; group g at col offset g * R * RS
        ring = const.tile([BO, B * R * RS], BF16, tag="ring")
        # per-group weight/bias tiles (shared when not merged)
        wtap, wprev0, w2t4, biast, scalet = {}, {}, {}, {}, {}
        for gw in range(NW):
            wtap[gw] = const.tile([BO, G4], BF16, tag=f"wtap{gw}",
                                  name=f"wtap{gw}")
            wprev0[gw] = const.tile([BO, G4], BF16, tag=f"wprev0{gw}",
                                    name=f"wprev0{gw}")
            w2t4[gw] = const.tile([CIN, G4], BF16, tag=f"w2t4{gw}",
                                  name=f"w2t4{gw}")
            biast[gw] = const.tile([G4, 1], F32, tag=f"biast{gw}",
                                   name=f"biast{gw}")
            scalet[gw] = const.tile([G4, 1], F32, tag=f"scalet{gw}",
                                    name=f"scalet{gw}")
        for g in range(B):
            gw = g % NW
            wtap[g], wprev0[g], w2t4[g] = wtap[gw], wprev0[gw], w2t4[gw]
            biast[g], scalet[g] = biast[gw], scalet[gw]
        zb = const.tile([G4, 1], F32, tag="zb")
        dummy = const.tile([BO, 4], CELLDT, tag="dummy")
        # gate-quadrant bases per group.  Non-merged: both groups use
        # (o,f,i,gg).  Merged: A=(o,i,f,gg), B=(i,o,gg,f) so the two
        # groups' cell rows land adjacent at partitions 64:128 (APs may
        # span 64 partitions only from base 0 or 64).
        if merged:
            O0 = {0: 0, 1: BO}
            I0 = {0: BO, 1: 0}
            F0 = {0: 2 * BO, 1: 3 * BO}
            GG0 = {0: 3 * BO, 1: 2 * BO}
            chAB = const.tile([G4, H], CELLDT, tag="chAB")
            chs = {g: chAB[F0[g] : F0[g] + BO, :] for g in range(B)}
        else:
            O0 = {0: 0, 1: 0}
            F0 = {0: BO, 1: BO}
            I0 = {0: 2 * BO, 1: 2 * BO}
            GG0 = {0: 3 * BO, 1: 3 * BO}
            ch = {g: const.tile([2 * BO, H], CELLDT, tag=f"ch{g}",
                                name=f"ch{g}")
                  for g in range(B)}
            chs = {g: ch[g][BO : 2 * BO, :] for g in range(B)}

        # ---- preamble ----
        nc.vector.memset(zb[:, :], 0.0)
        nc.vector.memset(zl1[:, :], 0.0)
        nc.vector.memset(zq[:, :], 0.0)
        # dummy activation: pulls the act-table load (~1.3us) into the
        # DMA wait instead of the first real sigmoid
        nc.scalar.activation(
            dummy[:, :], zq[:, 0:4], AF.Sigmoid, bias=zb[0:BO, :], scale=1.0
        )
        for gw in range(NW):
            o = gw * 2 * G4
            nc.sync.dma_start(
                out=wtap[gw][:, :], in_=wpack_d.ap()[:, o : o + G4]
            )
            nc.sync.dma_start(
                out=wprev0[gw][:, :],
                in_=wpack_d.ap()[:, o + G4 : o + 2 * G4],
            )
            nc.sync.dma_start(
                out=w2t4[gw][:, :],
                in_=w2t_d.ap()[:, gw * G4 : (gw + 1) * G4],
            )
            nc.sync.dma_start(
                out=biast[gw][:, :], in_=bsc_d.ap()[:, 2 * gw : 2 * gw + 1]
            )
            nc.sync.dma_start(
                out=scalet[gw][:, :],
                in_=bsc_d.ap()[:, 2 * gw + 1 : 2 * gw + 2],
            )
            if not qsplit:
                # legacy sigmoid(2z) trick wants the gg bias doubled
                gg0 = GG0[gw] if merged else 3 * BO
                nc.vector.tensor_scalar(
                    biast[gw][gg0 : gg0 + BO, :],
                    biast[gw][gg0 : gg0 + BO, :],
                    2.0, None, ALU.mult,
                )
        # x in 16-row chunks: lo halves (rows 0-63) first -- the hi halves
        # are first read at t=64, ~190us into the scan (the SP sequencer
        # issues DMAs serially at ~650ns each; issue order gates startup)
        for q in range(4):
            for blk in (0, 2):
                c0 = blk * 64 * T + q * 16 * T
                nc.sync.dma_start(
                    out=xc[:, c0 : c0 + 16 * T],
                    in_=xc_d.ap()[:, c0 : c0 + 16 * T],
                )
        for q in range(4):
            for blk in (1, 3):
                c0 = blk * 64 * T + q * 16 * T
                nc.sync.dma_start(
                    out=xc[:, c0 : c0 + 16 * T],
                    in_=xc_d.ap()[:, c0 : c0 + 16 * T],
                )

        # slot R-1 (read by step 0's taps) first, then the rest
        for g in range(B):
            o = g * R * RS
            nc.vector.memset(ring[:, o + (R - 1) * RS : o + R * RS], 0.0)
        for g in range(B):
            o = g * R * RS
            nc.vector.memset(ring[:, o : o + (R - 1) * RS], 0.0)
        if merged:
            nc.vector.memset(chAB[:, :], 0.0)
        else:
            for g in range(B):
                nc.vector.memset(ch[g][:, :], 0.0)

        rv = ring[:, :].rearrange("p (g s c) -> p g s c", g=B, s=R)
        xv = xc[:, :].rearrange("p (b c) -> p b c", b=4)  # 4 column blocks

        # ---- the scan ----
        import contextlib

        rep_ctx = tc.For_i(0, reps, 1) if reps > 1 else contextlib.nullcontext()
        with rep_ctx:
            for t in range(TS):
                sp = (t - 1) % R
                sl = t % R

                def mm_phase(g):
                    gp = psum.tile([G4, H], F32, tag=f"g{g}", name=f"g{g}")
                    # PSUM init: contraction-1 zero matmul (real-HW MM
                    # time scales with contraction via weight load; a
                    # 32-row zero-mm cost 260ns of PE occupancy that
                    # could collide with the chain-critical taps)
                    nc.tensor.matmul(
                        gp[:, :], zl1[:, :], zq[0:1, :],
                        start=True, stop=False,
                    )
                    # x-term: valid rows only (diagonal stride-127 reads);
                    # off the serial chain (depends only on the input DMA)
                    if not no_x:
                        nlo = min(t + 1, 64)
                        nc.tensor.matmul(
                            gp[:, 0:nlo], w2t4[g][:, :],
                            xv[:, 2 * g, t : t + 127 * (nlo - 1) + 1 : 127],
                            start=False, stop=False,
                        )
                        if t >= 64:
                            nhi = t - 64 + 1
                            d0 = t - 64
                            nc.tensor.matmul(
                                gp[:, 64 : 64 + nhi], w2t4[g][:, :],
                                xv[:, 2 * g + 1,
                                   d0 : d0 + 127 * (nhi - 1) + 1 : 127],
                                start=False, stop=False,
                            )
                    # state taps (the only mms on the serial chain).
                    # The h-wait rides the lowered Ldweights, so an
                    # h-dependent tap starts cold (~150ns weight fill on
                    # the chain).  Warm-up: a 1-col matmul with rhs from
                    # zq (constant zeros, preamble-written, no per-step
                    # deps at all) runs during the h-wait, pre-loading
                    # wtap into the PE array and accumulating exactly 0;
                    # the real tap then starts warm (~108ns).
                    nc.tensor.matmul(
                        gp[:, 0:1], wtap[g][:, :],
                        zq[:, 0:1],
                        start=False, stop=False,
                    )
                    nc.tensor.matmul(
                        gp[:, :], wtap[g][:, :],
                        rv[0:BO, g, sp, 1 : 1 + H],
                        start=False, stop=False,
                    )
                    nc.tensor.matmul(
                        gp[:, :], wprev0[g][:, :],
                        rv[0:BO, g, sp, 0:H],
                        start=False, stop=True,
                    )
                    return gp

                gps = [mm_phase(g) for g in range(B)]

                sg, w, t1, tg = {}, {}, {}, {}
                tchs = {}
                for g in range(B):
                    sg[g] = sig_p.tile([G4, H], CELLDT, tag=f"sg{g}",
                                       name=f"sg{g}")
                    if qsplit:
                        # sigmoid on o,f,i quadrants only (scale folded to
                        # 1.0: no scale operand fetch)
                        nc.scalar.activation(
                            sg[g][0 : 3 * BO, :], gps[g][0 : 3 * BO, :],
                            AF.Sigmoid, bias=biast[g][0 : 3 * BO, :],
                        )
                        # gg quadrant: direct tanh, realigned to the
                        # i-quadrant base so the u4 stt sees equal bases
                        tg[g] = tmp_p.tile(
                            [G4, H], CELLDT, tag=f"tg{g}",
                            name=f"tg{g}")[I0[g] : I0[g] + BO, :]
                        nc.scalar.activation(
                            tg[g], gps[g][3 * BO : 4 * BO, :],
                            AF.Tanh, bias=biast[g][3 * BO : 4 * BO, :],
                        )
                    else:
                        nc.scalar.activation(
                            sg[g][:, :], gps[g][:, :], AF.Sigmoid,
                            bias=biast[g][:, :],
                            scale=1.0 if pre2 else scalet[g][:, :],
                        )
                if no_cell:
                    for g in range(B):
                        # timing ablation: ring write straight from sg
                        nc.vector.scalar_tensor_tensor(
                            rv[0:BO, g, sl, 1 : 1 + H],
                            sg[g][O0[g] : O0[g] + BO, :],
                            0.5, sg[g][O0[g] : O0[g] + BO, :],
                            ALU.subtract, ALU.mult,
                        )
                    continue
                # t1 on Pool: all groups when not t1_dve, or only the
                # groups selected by the t1_split bitmask (Pool is idle;
                # one Pool t1 avoids the both-on-Pool queueing loss)
                t1_on_pool = {g: (not t1_dve) or bool(t1_split & (1 << g))
                              for g in range(B)}
                for g in range(B):
                    if t1_on_pool[g]:
                        t1[g] = tmp_p.tile(
                            [G4, H], CELLDT, tag=f"t1{g}",
                            name=f"t1{g}")[F0[g] : F0[g] + BO, :]
                        nc.gpsimd.tensor_tensor(
                            t1[g], sg[g][F0[g] : F0[g] + BO, :],
                            chs[g], ALU.mult,
                        )
                # DVE trio per group, group-major so the other group's ops
                # don't interleave into this group's chain
                for g in range(B):
                    if g not in w:
                        w[g] = tmp_p.tile([G4, H], CELLDT, tag=f"w{g}",
                                          name=f"w{g}")
                    if qsplit:
                        if t1_dve:
                            # t1 = sig_f * CH4
                            t1[g] = tmp_p.tile(
                                [G4, H], CELLDT, tag=f"t1{g}",
                                name=f"t1{g}")[F0[g] : F0[g] + BO, :]
                            nc.vector.tensor_tensor(
                                t1[g], sg[g][F0[g] : F0[g] + BO, :],
                                chs[g], ALU.mult,
                            )
                        # u4 = (tanh_gg * 2) * sig_i  (equal bases at I0)
                        nc.vector.scalar_tensor_tensor(
                            w[g][F0[g] : F0[g] + BO, :], tg[g], 2.0,
                            sg[g][I0[g] : I0[g] + BO, :],
                            ALU.mult, ALU.mult,
                        )
                        eng_add = nc.gpsimd if pool_add else nc.vector
                        eng_add.tensor_tensor(
                            chs[g], w[g][F0[g] : F0[g] + BO, :],
                            t1[g], ALU.add,
                        )
                        continue
                    def _cgs():
                        # cgs4 = 4*sigmoid(2g_gg) - 2 = 2*tanh(g_gg),
                        # realigned from the gg quadrant to the i base
                        nc.vector.tensor_scalar(
                            w[g][I0[g] : I0[g] + BO, :],
                            sg[g][GG0[g] : GG0[g] + BO, :],
                            4.0, 2.0, ALU.mult, ALU.subtract,
                        )

                    def _t1():
                        # t1 = sig_f * CH4 (fills the cgs->u4 RAW-drain
                        # bubble when second)
                        t1[g] = tmp_p.tile(
                            [G4, H], CELLDT, tag=f"t1{g}",
                            name=f"t1{g}")[F0[g] : F0[g] + BO, :]
                        nc.vector.tensor_tensor(
                            t1[g], sg[g][F0[g] : F0[g] + BO, :],
                            chs[g], ALU.mult,
                        )

                    if t1_first and not t1_on_pool[g]:
                        _t1(); _cgs()
                    else:
                        _cgs()
                        if not t1_on_pool[g]:
                            _t1()
                    # u4 = cgs4 * sig_i = 2*i*gg -> the f-quadrant base
                    nc.vector.tensor_tensor(
                        w[g][F0[g] : F0[g] + BO, :],
                        w[g][I0[g] : I0[g] + BO, :],
                        sg[g][I0[g] : I0[g] + BO, :], ALU.mult,
                    )
                    # CH4 = u4 + t1 (queue-contiguous: no sem hop on chain)
                    eng_add = nc.gpsimd if pool_add else nc.vector
                    eng_add.tensor_tensor(
                        chs[g], w[g][F0[g] : F0[g] + BO, :],
                        t1[g], ALU.add,
                    )
                if no_tail:
                    for g in range(B):
                        # timing ablation: ring write from CH4, no sigma4
                        nc.vector.scalar_tensor_tensor(
                            rv[0:BO, g, sl, 1 : 1 + H],
                            chs[g], 0.5, t1[g],
                            ALU.subtract, ALU.mult,
                        )
                    continue
                # tch = tanh(0.5*CH4) = tanh(c) (same act table as
                # Sigmoid: sigmoid_and_others has both -> no reload)
                if merged:
                    # both groups' cell rows are adjacent (64:128): ONE
                    # tanh op + one sem instead of two
                    tchAB = tmp_p.tile([2 * BO, H], CELLDT, tag="tchAB",
                                       name="tchAB")
                    nc.scalar.activation(
                        tchAB[:, :], chAB[2 * BO : 4 * BO, :], AF.Tanh,
                        bias=zb[2 * BO : 4 * BO, :], scale=0.5,
                    )
                    for g in range(B):
                        tchs[g] = tchAB[F0[g] - 2 * BO :
                                        F0[g] - 2 * BO + BO, :]
                else:
                    for g in range(B):
                        tch_t = tmp_p.tile([BO, H], CELLDT, tag=f"tch{g}",
                                           name=f"tch{g}")
                        nc.scalar.activation(
                            tch_t[:, :], chs[g], AF.Tanh,
                            bias=zb[BO : 2 * BO, :], scale=0.5,
                        )
                        tchs[g] = tch_t[:, :]
                for g in range(B):
                    # sh = tch * sig_o = o*tanh(c) = h -> ring hA
                    eng_h = nc.vector if h_dve else nc.gpsimd
                    eng_h.tensor_tensor(
                        rv[0:BO, g, sl, 1 : 1 + H], tchs[g],
                        sg[g][O0[g] : O0[g] + BO, :], ALU.mult,
                    )

                if t % CHUNK == CHUNK - 1 and not no_out:
                    # contiguous SBUF source (1 descriptor per partition)
                    c0 = t - CHUNK + 1
                    s0 = c0 % R
                    for g in range(B):
                        o0 = (g * R + s0) * RS
                        nc.sync.dma_start(
                            out=hs_d.ap()[g, t // CHUNK, :, :],
                            in_=ring[0:BO, o0 : o0 + CHUNK * RS],
                        )

        if timing:
            # tiny real output so the bass_exec call cannot be elided
            tt = const.tile([1, 4], F32, tag="tt")
            nc.vector.tensor_copy(tt[:, :], ring[0:1, 0:4])
            nc.sync.dma_start(out=tiny_d.ap(), in_=tt[:, :])

    nc.compile()
    return nc


_NC_CACHE = {}


def _get_module(**kw):
    key = tuple(sorted(kw.items()))
    if key not in _NC_CACHE:
        _NC_CACHE[key] = _build_module(**kw)
    return _NC_CACHE[key]


def _prep_weights(W2, b2, W1, b1, merged=False, pre2=False):
    import ml_dtypes

    W2 = np.asarray(W2, np.float32)
    W1 = np.asarray(W1, np.float32)
    b = np.asarray(b1, np.float32) + np.asarray(b2, np.float32)
    bias = b.copy()  # plain; legacy sigmoid(2z) path doubles gg on device
    scale = np.ones(G4, np.float32)
    scale[3 * BO :] = 2.0
    bf = ml_dtypes.bfloat16
    wcur = np.ascontiguousarray(W1[:, :, 1].T).astype(bf)
    wprev = np.ascontiguousarray(W1[:, :, 0].T).astype(bf)
    w2t = np.ascontiguousarray(W2.T).astype(bf)
    # gate orders (source layout is (o,f,i,gg)):
    #   non-merged: identity for both groups
    #   merged:     A=(o,i,f,gg), B=(i,o,gg,f)
    o_b, f_b, i_b, g_b = (np.arange(k * BO, (k + 1) * BO) for k in range(4))
    if merged:
        perms = [
            np.concatenate([o_b, i_b, f_b, g_b]),
            np.concatenate([i_b, o_b, g_b, f_b]),
        ]
    else:
        perms = [np.arange(G4)]
    if pre2:
        # pre-double the gg columns so sigma needs no scale operand
        # (the sigmoid(2z) trick's 2x rides in the weights)
        for m in (wcur, wprev, w2t):
            m[:, 3 * BO :] = (m[:, 3 * BO :].astype(np.float32) * 2.0
                              ).astype(m.dtype)
    wps, w2s, bscs = [], [], []
    for p in perms:
        wps += [wcur[:, p], wprev[:, p]]
        w2s.append(w2t[:, p])
        bscs += [bias[p], scale[p]]
    wpack = np.ascontiguousarray(np.concatenate(wps, axis=1))
    w2tp = np.ascontiguousarray(np.concatenate(w2s, axis=1))
    bsc = np.ascontiguousarray(np.stack(bscs, axis=1))
    return wpack, w2tp, bsc


def _prep_canvas(x):
    """Per-core natural-layout x [CIN, 4*64*T] bf16, partitions 0-31,
    column blocks (g0 rows 0-63, g0 rows 64-127, g1 lo, g1 hi)."""
    import ml_dtypes

    nb, _, _, _ = x.shape  # (16, CIN, H, T)
    out = np.empty((nb // B, CIN, 4 * 64 * T), np.float32)
    for k in range(nb // B):
        for g in range(B):
            xb = x[B * k + g]  # (CIN, H, T)
            o = 2 * g * 64 * T
            out[k, :, o : o + 64 * T] = xb[:, 0:64, :].reshape(CIN, -1)
            out[k, :, o + 64 * T : o + 2 * 64 * T] = (
                xb[:, 64:128, :].reshape(CIN, -1)
            )
    return out.astype(ml_dtypes.bfloat16)


def kernel(x, W2, b2, W1, b1):
    from concourse.bass_utils import run_bass_kernel_spmd

    nc = _get_module(merged=MERGED, pre2=PRE2)
    x = np.ascontiguousarray(x, dtype=np.float32)
    wpack, w2t, bsc = _prep_weights(W2, b2, W1, b1, merged=MERGED,
                                    pre2=PRE2)
    xcs = _prep_canvas(x)
    in_maps = [
        {"xc": xcs[k], "wpack": wpack, "w2t": w2t, "bsc": bsc}
        for k in range(N_CORES)
    ]
    res = run_bass_kernel_spmd(nc, in_maps, list(range(N_CORES)))
    out = np.empty((N_CORES * B, BO, H, T), np.float32)
    for k in range(N_CORES):
        hs = _decode_hs(res.results[k]["hs"])
        out[2 * k : 2 * k + 2] = hs
    return out


def _decode_hs(hs):
    """(B, T//CHUNK, BO, CHUNK*RS) raw ring dump -> (B, BO, H, T) = 2*SH."""
    hs = np.asarray(hs, np.float32).reshape(B, T // CHUNK, BO, CHUNK, RS)
    hs = hs[:, :, :, :, 1 : 1 + H]  # (B, nch, BO, CHUNK, H=row)
    hs = hs.transpose(0, 2, 4, 1, 3).reshape(B, BO, H, T)
    return hs



# revision 65
# speedup vs baseline: 1.0154x; 1.0092x over previous
"""Bass/Tile TRN2 kernel for nn_DiagonalLSTM (v4).

Data-parallel over batch: 16 batch elements across 8 cores -> 2 per core.
Per core, two independent pipelines ("groups", one per batch element)
run the 128-step LSTM scan.  The serial per-step dependency chain - not
engine throughput - bounds the runtime, so the design minimizes it:

  sh(t-1) -> taps-mms -> sigmoid(gates) -> [cgs4, t1, u4, add] DVE block
          -> tanh(c) -> sh(t)

- PSUM is initialized by a dependency-free zero matmul, and the x-term
  matmuls (valid rows only, stride-127 diagonal reads of natural-layout
  bf16 x) depend only on the input DMA: both run ahead; only the two
  bf16 state-tap matmuls sit on the chain (real NTFF trace: 260ns
  cold-weights + 108ns warm, LDWEIGHTS ~100ns pipelined alongside).
- The prev-tap (row shift by one) is the same ring slot read one column
  earlier; column 0 of each slot is a permanent zero pad.
- All matmul operands live at partition base 0: members of one PSUM
  accumulation group must share a PE row band (mixed tile_positions
  crash the device).
- One act table for everything: the sigmoid_and_others set holds both
  Sigmoid and Tanh; a dummy preamble activation pulls the ~1.3us table
  load into the input-DMA wait.  c-state kept as CH4 = 2c; candidate
  gate via cgs4 = 4*sigmoid(2g)-2 = 2*tanh(g) with one dual-scalar
  tensor_scalar; the 2x inside sigmoid(2g) is pre-scaled into the gg
  weight columns on the host (pre2) so sigma needs no per-partition
  scale operand (measured ~85ns/step on HW); the ring stores full
  h = tch * sig_o.
- Engine placement (real-NTFF-measured; the chain is ENGINE-EXEC
  bound: DVE ~214ns/op, Sigmoid 367/Tanh 400, sem gaps only 10-90ns
  -- the CoreSim cost model has this backwards): the WHOLE
  elementwise cell runs on DVE - per group a
  queue-contiguous block [cgs4, t1, u4, add] (cgs first, t1 second
  fills the cgs->u4 RAW-drain bubble) and the final h-multiply.  Pool
  (gpsimd) is ~430ns/op loaded vs DVE ~375 and lost every HW A/B race.
  The cgs realign is forced by the verifier's equal-base rule for
  2-input DVE ops (cross-quadrant operands reject or corrupt).  bf16
  cell tiles enable the DVE 2x perf mode.
- Startup: x DMAs issue lo-row-halves first (hi halves are first read
  at t=64, ~190us in); SP issues serially at ~650ns each so issue
  order gates the scan start.
- Dead ends measured on HW, kept out: Pool for t1/h (slower per op);
  qsplit (direct-Tanh gg: +1 ACT op serializes what the -1 DVE op
  saves); merged two-group tanh (lockstep coupling costs ~80us);
  affine_mul_reduce custom-DVE cgs+u4 fusion (cross-base operands
  corrupt on HW); packed weight/bias tiles as SBUF operands (sliced /
  strided engine operands cost ~600ns/step; packed DRAM with sliced
  DMA reads into dedicated tiles is fine).
- The whole cell is bf16 except biases; measured rel err 9.7e-3 on HW
  (gate 2e-2, deterministic inputs).
- Output is a raw ring dump every CHUNK=8 steps (contiguous SBUF DMA),
  decoded on the host.
- timing=True builds declare all data tensors Internal so repeated
  calls move no host bytes (used by test.py's interleaved differencing).
"""

import sys

sys.path.insert(0, "/opt/trn_rl_repo")

from contextlib import ExitStack

import numpy as np

import concourse.bass as bass
import concourse.tile as tile
from concourse import bacc, mybir

F32 = mybir.dt.float32
BF16 = mybir.dt.bfloat16
AF = mybir.ActivationFunctionType
ALU = mybir.AluOpType

N_CORES = 8
B = 2  # batch per core (= groups)
CIN = 32
H = 128  # rows
T = 128  # scan steps
BO = 32
G4 = 4 * BO  # gate channels, partition order (o, f, i, gg)
RS = 130  # ring slot columns: [pad, 128 rows, spare]
R = 16  # ring depth (slots)
CHUNK = 8  # output DMA chunk (steps); divides T, <= R/2
LOOSE_U = False  # the HW birverifier (NCC_IBIR297) demands equal base
# partitions for 2-input SBUF DVE ops, so u needs the cgs realign first
CELLDT = BF16  # cell-state dtype: bf16 enables the DVE 2x perf mode
MERGED = False  # merged-tanh variant (group-B gates reordered)
PRE2 = True  # gg 2x pre-scaled into weights: sigma needs no scale operand


def _build_module(reps=1, t_steps=None, no_out=False, timing=False,
                  no_x=False, no_tail=False, no_cell=False,
                  pool_add=False, t1_dve=True, h_dve=True,
                  qsplit=False, bufs=2, psum_bufs=2, merged=False,
                  pre2=True, t1_first=False, t1_split=0):
    TS = T if t_steps is None else t_steps
    nc = bacc.Bacc(
        "TRN2",
        target_bir_lowering=False,
        debug=False,
        num_devices=N_CORES,
    )

    # timing builds take no external data (uninitialized internal DRAM;
    # instruction stream and therefore timing are identical) so repeated
    # calls move no host bytes
    ki = "Internal" if timing else "ExternalInput"
    ko = "Internal" if timing else "ExternalOutput"
    # natural-layout x, bf16, all at partitions 0-31 (cin), 4 column
    # blocks of 64*T: (g0 rows 0-63, g0 rows 64-127, g1 lo, g1 hi).
    # Matmuls may only mix within one PE row band: every matmul operand
    # lives at partition base 0 (mixed tile_positions in one PSUM
    # accumulation group crash the device).
    # merged: per-group gate orders -> per-group weight/bias copies
    NW = B if merged else 1
    xc_d = nc.dram_tensor("xc", [CIN, 4 * 64 * T], BF16, kind=ki)
    wpack_d = nc.dram_tensor("wpack", [BO, NW * 2 * G4], BF16, kind=ki)
    w2t_d = nc.dram_tensor("w2t", [CIN, NW * G4], BF16, kind=ki)
    bsc_d = nc.dram_tensor("bsc", [G4, NW * 2], F32, kind=ki)
    # raw ring dump: [group, chunk, 32 gates, CHUNK slots * RS cols]
    hs_d = nc.dram_tensor(
        "hs", [B, TS // CHUNK, BO, CHUNK * RS], BF16, kind=ko
    )
    tiny_d = (
        nc.dram_tensor("tiny", [1, 4], F32, kind="ExternalOutput")
        if timing else None
    )

    with ExitStack() as ctx:
        tc = ctx.enter_context(tile.TileContext(nc))
        const = ctx.enter_context(tc.tile_pool(name="const", bufs=1))
        psum = ctx.enter_context(
            tc.tile_pool(name="psum", bufs=psum_bufs, space="PSUM")
        )
        sig_p = ctx.enter_context(tc.tile_pool(name="sig", bufs=bufs))
        tmp_p = ctx.enter_context(tc.tile_pool(name="tmp", bufs=bufs))

        # ---- persistent tiles ----
        xc = const.tile([CIN, 4 * 64 * T], BF16, tag="xc")
        zq = const.tile([BO, H], BF16, tag="zq")  # zero matmul rhs
        zl1 = const.tile([1, G4], BF16, tag="zl1")  # contraction-1 zero lhsT
        # ring: h(r) at slot col 1+r; group g at col offset g * R * RS
        ring = const.tile([BO, B * R * RS], BF16, tag="ring")
        # per-group weight/bias tiles (shared when not merged)
        wtap, wprev0, w2t4, biast, scalet = {}, {}, {}, {}, {}
        for gw in range(NW):
            wtap[gw] = const.tile([BO, G4], BF16, tag=f"wtap{gw}",
                                  name=f"wtap{gw}")
            wprev0[gw] = const.tile([BO, G4], BF16, tag=f"wprev0{gw}",
                                    name=f"wprev0{gw}")
            w2t4[gw] = const.tile([CIN, G4], BF16, tag=f"w2t4{gw}",
                                  name=f"w2t4{gw}")
            biast[gw] = const.tile([G4, 1], F32, tag=f"biast{gw}",
                                   name=f"biast{gw}")
            scalet[gw] = const.tile([G4, 1], F32, tag=f"scalet{gw}",
                                    name=f"scalet{gw}")
        for g in range(B):
            gw = g % NW
            wtap[g], wprev0[g], w2t4[g] = wtap[gw], wprev0[gw], w2t4[gw]
            biast[g], scalet[g] = biast[gw], scalet[gw]
        zb = const.tile([G4, 1], F32, tag="zb")
        dummy = const.tile([BO, 4], CELLDT, tag="dummy")
        # gate-quadrant bases per group.  Non-merged: both groups use
        # (o,f,i,gg).  Merged: A=(o,i,f,gg), B=(i,o,gg,f) so the two
        # groups' cell rows land adjacent at partitions 64:128 (APs may
        # span 64 partitions only from base 0 or 64).
        if merged:
            O0 = {0: 0, 1: BO}
            I0 = {0: BO, 1: 0}
            F0 = {0: 2 * BO, 1: 3 * BO}
            GG0 = {0: 3 * BO, 1: 2 * BO}
            chAB = const.tile([G4, H], CELLDT, tag="chAB")
            chs = {g: chAB[F0[g] : F0[g] + BO, :] for g in range(B)}
        else:
            O0 = {0: 0, 1: 0}
            F0 = {0: BO, 1: BO}
            I0 = {0: 2 * BO, 1: 2 * BO}
            GG0 = {0: 3 * BO, 1: 3 * BO}
            ch = {g: const.tile([2 * BO, H], CELLDT, tag=f"ch{g}",
                                name=f"ch{g}")
                  for g in range(B)}
            chs = {g: ch[g][BO : 2 * BO, :] for g in range(B)}

        # ---- preamble ----
        nc.vector.memset(zb[:, :], 0.0)
        nc.vector.memset(zl1[:, :], 0.0)
        nc.vector.memset(zq[:, :], 0.0)
        # dummy activation: pulls the act-table load (~1.3us) into the
        # DMA wait instead of the first real sigmoid
        nc.scalar.activation(
            dummy[:, :], zq[:, 0:4], AF.Sigmoid, bias=zb[0:BO, :], scale=1.0
        )
        for gw in range(NW):
            o = gw * 2 * G4
            nc.sync.dma_start(
                out=wtap[gw][:, :], in_=wpack_d.ap()[:, o : o + G4]
            )
            nc.sync.dma_start(
                out=wprev0[gw][:, :],
                in_=wpack_d.ap()[:, o + G4 : o + 2 * G4],
            )
            nc.sync.dma_start(
                out=w2t4[gw][:, :],
                in_=w2t_d.ap()[:, gw * G4 : (gw + 1) * G4],
            )
            nc.sync.dma_start(
                out=biast[gw][:, :], in_=bsc_d.ap()[:, 2 * gw : 2 * gw + 1]
            )
            nc.sync.dma_start(
                out=scalet[gw][:, :],
                in_=bsc_d.ap()[:, 2 * gw + 1 : 2 * gw + 2],
            )
            if not qsplit:
                # legacy sigmoid(2z) trick wants the gg bias doubled
                gg0 = GG0[gw] if merged else 3 * BO
                nc.vector.tensor_scalar(
                    biast[gw][gg0 : gg0 + BO, :],
                    biast[gw][gg0 : gg0 + BO, :],
                    2.0, None, ALU.mult,
                )
        # x in 16-row chunks: lo halves (rows 0-63) first -- the hi halves
        # are first read at t=64, ~190us into the scan (the SP sequencer
        # issues DMAs serially at ~650ns each; issue order gates startup)
        for q in range(4):
            for blk in (0, 2):
                c0 = blk * 64 * T + q * 16 * T
                nc.sync.dma_start(
                    out=xc[:, c0 : c0 + 16 * T],
                    in_=xc_d.ap()[:, c0 : c0 + 16 * T],
                )
        for q in range(4):
            for blk in (1, 3):
                c0 = blk * 64 * T + q * 16 * T
                nc.sync.dma_start(
                    out=xc[:, c0 : c0 + 16 * T],
                    in_=xc_d.ap()[:, c0 : c0 + 16 * T],
                )

        # slot R-1 (read by step 0's taps) first, then the rest
        for g in range(B):
            o = g * R * RS
            nc.vector.memset(ring[:, o + (R - 1) * RS : o + R * RS], 0.0)
        for g in range(B):
            o = g * R * RS
            nc.vector.memset(ring[:, o : o + (R - 1) * RS], 0.0)
        if merged:
            nc.vector.memset(chAB[:, :], 0.0)
        else:
            for g in range(B):
                nc.vector.memset(ch[g][:, :], 0.0)

        rv = ring[:, :].rearrange("p (g s c) -> p g s c", g=B, s=R)
        xv = xc[:, :].rearrange("p (b c) -> p b c", b=4)  # 4 column blocks

        # ---- the scan ----
        import contextlib

        rep_ctx = tc.For_i(0, reps, 1) if reps > 1 else contextlib.nullcontext()
        with rep_ctx:
            for t in range(TS):
                sp = (t - 1) % R
                sl = t % R

                def mm_phase(g):
                    gp = psum.tile([G4, H], F32, tag=f"g{g}", name=f"g{g}")
                    # PSUM init: contraction-1 zero matmul (real-HW MM
                    # time scales with contraction via weight load; a
                    # 32-row zero-mm cost 260ns of PE occupancy that
                    # could collide with the chain-critical taps)
                    nc.tensor.matmul(
                        gp[:, :], zl1[:, :], zq[0:1, :],
                        start=True, stop=False,
                    )
                    # x-term: valid rows only (diagonal stride-127 reads);
                    # off the serial chain (depends only on the input DMA)
                    if not no_x:
                        nlo = min(t + 1, 64)
                        nc.tensor.matmul(
                            gp[:, 0:nlo], w2t4[g][:, :],
                            xv[:, 2 * g, t : t + 127 * (nlo - 1) + 1 : 127],
                            start=False, stop=False,
                        )
                        if t >= 64:
                            nhi = t - 64 + 1
                            d0 = t - 64
                            nc.tensor.matmul(
                                gp[:, 64 : 64 + nhi], w2t4[g][:, :],
                                xv[:, 2 * g + 1,
                                   d0 : d0 + 127 * (nhi - 1) + 1 : 127],
                                start=False, stop=False,
                            )
                    # state taps (the only mms on the serial chain).
                    # tap1 split 32+96 cols: the h-wait rides the lowered
                    # Ldweights, so the first MM always starts cold
                    # (~260ns); a small cold slice + large warm slice
                    # (LDW pipelines under the running MM) is cheaper.
                    nc.tensor.matmul(
                        gp[:, 0:32], wtap[g][:, :],
                        rv[0:BO, g, sp, 1:33],
                        start=False, stop=False,
                    )
                    nc.tensor.matmul(
                        gp[:, 32:H], wtap[g][:, :],
                        rv[0:BO, g, sp, 33 : 1 + H],
                        start=False, stop=False,
                    )
                    nc.tensor.matmul(
                        gp[:, :], wprev0[g][:, :],
                        rv[0:BO, g, sp, 0:H],
                        start=False, stop=True,
                    )
                    return gp

                gps = [mm_phase(g) for g in range(B)]

                sg, w, t1, tg = {}, {}, {}, {}
                tchs = {}
                for g in range(B):
                    sg[g] = sig_p.tile([G4, H], CELLDT, tag=f"sg{g}",
                                       name=f"sg{g}")
                    if qsplit:
                        # sigmoid on o,f,i quadrants only (scale folded to
                        # 1.0: no scale operand fetch)
                        nc.scalar.activation(
                            sg[g][0 : 3 * BO, :], gps[g][0 : 3 * BO, :],
                            AF.Sigmoid, bias=biast[g][0 : 3 * BO, :],
                        )
                        # gg quadrant: direct tanh, realigned to the
                        # i-quadrant base so the u4 stt sees equal bases
                        tg[g] = tmp_p.tile(
                            [G4, H], CELLDT, tag=f"tg{g}",
                            name=f"tg{g}")[I0[g] : I0[g] + BO, :]
                        nc.scalar.activation(
                            tg[g], gps[g][3 * BO : 4 * BO, :],
                            AF.Tanh, bias=biast[g][3 * BO : 4 * BO, :],
                        )
                    else:
                        nc.scalar.activation(
                            sg[g][:, :], gps[g][:, :], AF.Sigmoid,
                            bias=biast[g][:, :],
                            scale=1.0 if pre2 else scalet[g][:, :],
                        )
                if no_cell:
                    for g in range(B):
                        # timing ablation: ring write straight from sg
                        nc.vector.scalar_tensor_tensor(
                            rv[0:BO, g, sl, 1 : 1 + H],
                            sg[g][O0[g] : O0[g] + BO, :],
                            0.5, sg[g][O0[g] : O0[g] + BO, :],
                            ALU.subtract, ALU.mult,
                        )
                    continue
                # t1 on Pool: all groups when not t1_dve, or only the
                # groups selected by the t1_split bitmask (Pool is idle;
                # one Pool t1 avoids the both-on-Pool queueing loss)
                t1_on_pool = {g: (not t1_dve) or bool(t1_split & (1 << g))
                              for g in range(B)}
                for g in range(B):
                    if t1_on_pool[g]:
                        t1[g] = tmp_p.tile(
                            [G4, H], CELLDT, tag=f"t1{g}",
                            name=f"t1{g}")[F0[g] : F0[g] + BO, :]
                        nc.gpsimd.tensor_tensor(
                            t1[g], sg[g][F0[g] : F0[g] + BO, :],
                            chs[g], ALU.mult,
                        )
                # DVE trio per group, group-major so the other group's ops
                # don't interleave into this group's chain
                for g in range(B):
                    if g not in w:
                        w[g] = tmp_p.tile([G4, H], CELLDT, tag=f"w{g}",
                                          name=f"w{g}")
                    if qsplit:
                        if t1_dve:
                            # t1 = sig_f * CH4
                            t1[g] = tmp_p.tile(
                                [G4, H], CELLDT, tag=f"t1{g}",
                                name=f"t1{g}")[F0[g] : F0[g] + BO, :]
                            nc.vector.tensor_tensor(
                                t1[g], sg[g][F0[g] : F0[g] + BO, :],
                                chs[g], ALU.mult,
                            )
                        # u4 = (tanh_gg * 2) * sig_i  (equal bases at I0)
                        nc.vector.scalar_tensor_tensor(
                            w[g][F0[g] : F0[g] + BO, :], tg[g], 2.0,
                            sg[g][I0[g] : I0[g] + BO, :],
                            ALU.mult, ALU.mult,
                        )
                        eng_add = nc.gpsimd if pool_add else nc.vector
                        eng_add.tensor_tensor(
                            chs[g], w[g][F0[g] : F0[g] + BO, :],
                            t1[g], ALU.add,
                        )
                        continue
                    def _cgs():
                        # cgs4 = 4*sigmoid(2g_gg) - 2 = 2*tanh(g_gg),
                        # realigned from the gg quadrant to the i base
                        nc.vector.tensor_scalar(
                            w[g][I0[g] : I0[g] + BO, :],
                            sg[g][GG0[g] : GG0[g] + BO, :],
                            4.0, 2.0, ALU.mult, ALU.subtract,
                        )

                    def _t1():
                        # t1 = sig_f * CH4 (fills the cgs->u4 RAW-drain
                        # bubble when second)
                        t1[g] = tmp_p.tile(
                            [G4, H], CELLDT, tag=f"t1{g}",
                            name=f"t1{g}")[F0[g] : F0[g] + BO, :]
                        nc.vector.tensor_tensor(
                            t1[g], sg[g][F0[g] : F0[g] + BO, :],
                            chs[g], ALU.mult,
                        )

                    if t1_first and not t1_on_pool[g]:
                        _t1(); _cgs()
                    else:
                        _cgs()
                        if not t1_on_pool[g]:
                            _t1()
                    # u4 = cgs4 * sig_i = 2*i*gg -> the f-quadrant base
                    nc.vector.tensor_tensor(
                        w[g][F0[g] : F0[g] + BO, :],
                        w[g][I0[g] : I0[g] + BO, :],
                        sg[g][I0[g] : I0[g] + BO, :], ALU.mult,
                    )
                    # CH4 = u4 + t1 (queue-contiguous: no sem hop on chain)
                    eng_add = nc.gpsimd if pool_add else nc.vector
                    eng_add.tensor_tensor(
                        chs[g], w[g][F0[g] : F0[g] + BO, :],
                        t1[g], ALU.add,
                    )
                if no_tail:
                    for g in range(B):
                        # timing ablation: ring write from CH4, no sigma4
                        nc.vector.scalar_tensor_tensor(
                            rv[0:BO, g, sl, 1 : 1 + H],
                            chs[g], 0.5, t1[g],
                            ALU.subtract, ALU.mult,
                        )
                    continue
                # tch = tanh(0.5*CH4) = tanh(c) (same act table as
                # Sigmoid: sigmoid_and_others has both -> no reload)
                if merged:
                    # both groups' cell rows are adjacent (64:128): ONE
                    # tanh op + one sem instead of two
                    tchAB = tmp_p.tile([2 * BO, H], CELLDT, tag="tchAB",
                                       name="tchAB")
                    nc.scalar.activation(
                        tchAB[:, :], chAB[2 * BO : 4 * BO, :], AF.Tanh,
                        bias=zb[2 * BO : 4 * BO, :], scale=0.5,
                    )
                    for g in range(B):
                        tchs[g] = tchAB[F0[g] - 2 * BO :
                                        F0[g] - 2 * BO + BO, :]
                else:
                    for g in range(B):
                        tch_t = tmp_p.tile([BO, H], CELLDT, tag=f"tch{g}",
                                           name=f"tch{g}")
                        nc.scalar.activation(
                            tch_t[:, :], chs[g], AF.Tanh,
                            bias=zb[BO : 2 * BO, :], scale=0.5,
                        )
                        tchs[g] = tch_t[:, :]
                for g in range(B):
                    # sh = tch * sig_o = o*tanh(c) = h -> ring hA
                    eng_h = nc.vector if h_dve else nc.gpsimd
                    eng_h.tensor_tensor(
                        rv[0:BO, g, sl, 1 : 1 + H], tchs[g],
                        sg[g][O0[g] : O0[g] + BO, :], ALU.mult,
                    )

                if t % CHUNK == CHUNK - 1 and not no_out:
                    # contiguous SBUF source (1 descriptor per partition)
                    c0 = t - CHUNK + 1
                    s0 = c0 % R
                    for g in range(B):
                        o0 = (g * R + s0) * RS
                        nc.sync.dma_start(
                            out=hs_d.ap()[g, t // CHUNK, :, :],
                            in_=ring[0:BO, o0 : o0 + CHUNK * RS],
                        )

        if timing:
            # tiny real output so the bass_exec call cannot be elided
            tt = const.tile([1, 4], F32, tag="tt")
            nc.vector.tensor_copy(tt[:, :], ring[0:1, 0:4])
            nc.sync.dma_start(out=tiny_d.ap(), in_=tt[:, :])

    nc.compile()
    return nc


_NC_CACHE = {}


def _get_module(**kw):
    key = tuple(sorted(kw.items()))
    if key not in _NC_CACHE:
        _NC_CACHE[key] = _build_module(**kw)
    return _NC_CACHE[key]


def _prep_weights(W2, b2, W1, b1, merged=False, pre2=False):
    import ml_dtypes

    W2 = np.asarray(W2, np.float32)
    W1 = np.asarray(W1, np.float32)
    b = np.asarray(b1, np.float32) + np.asarray(b2, np.float32)
    bias = b.copy()  # plain; legacy sigmoid(2z) path doubles gg on device
    scale = np.ones(G4, np.float32)
    scale[3 * BO :] = 2.0
    bf = ml_dtypes.bfloat16
    wcur = np.ascontiguousarray(W1[:, :, 1].T).astype(bf)
    wprev = np.ascontiguousarray(W1[:, :, 0].T).astype(bf)
    w2t = np.ascontiguousarray(W2.T).astype(bf)
    # gate orders (source layout is (o,f,i,gg)):
    #   non-merged: identity for both groups
    #   merged:     A=(o,i,f,gg), B=(i,o,gg,f)
    o_b, f_b, i_b, g_b = (np.arange(k * BO, (k + 1) * BO) for k in range(4))
    if merged:
        perms = [
            np.concatenate([o_b, i_b, f_b, g_b]),
            np.concatenate([i_b, o_b, g_b, f_b]),
        ]
    else:
        perms = [np.arange(G4)]
    if pre2:
        # pre-double the gg columns so sigma needs no scale operand
        # (the sigmoid(2z) trick's 2x rides in the weights)
        for m in (wcur, wprev, w2t):
            m[:, 3 * BO :] = (m[:, 3 * BO :].astype(np.float32) * 2.0
                              ).astype(m.dtype)
    wps, w2s, bscs = [], [], []
    for p in perms:
        wps += [wcur[:, p], wprev[:, p]]
        w2s.append(w2t[:, p])
        bscs += [bias[p], scale[p]]
    wpack = np.ascontiguousarray(np.concatenate(wps, axis=1))
    w2tp = np.ascontiguousarray(np.concatenate(w2s, axis=1))
    bsc = np.ascontiguousarray(np.stack(bscs, axis=1))
    return wpack, w2tp, bsc


def _prep_canvas(x):
    """Per-core natural-layout x [CIN, 4*64*T] bf16, partitions 0-31,
    column blocks (g0 rows 0-63, g0 rows 64-127, g1 lo, g1 hi)."""
    import ml_dtypes

    nb, _, _, _ = x.shape  # (16, CIN, H, T)
    out = np.empty((nb // B, CIN, 4 * 64 * T), np.float32)
    for k in range(nb // B):
        for g in range(B):
            xb = x[B * k + g]  # (CIN, H, T)
            o = 2 * g * 64 * T
            out[k, :, o : o + 64 * T] = xb[:, 0:64, :].reshape(CIN, -1)
            out[k, :, o + 64 * T : o + 2 * 64 * T] = (
                xb[:, 64:128, :].reshape(CIN, -1)
            )
    return out.astype(ml_dtypes.bfloat16)


def kernel(x, W2, b2, W1, b1):
    from concourse.bass_utils import run_bass_kernel_spmd

    nc = _get_module(merged=MERGED, pre2=PRE2)
    x = np.ascontiguousarray(x, dtype=np.float32)
    wpack, w2t, bsc = _prep_weights(W2, b2, W1, b1, merged=MERGED,
                                    pre2=PRE2)
    xcs = _prep_canvas(x)
    in_maps = [
        {"xc": xcs[k], "wpack": wpack, "w2t": w2t, "bsc": bsc}
        for k in range(N_CORES)
    ]
    res = run_bass_kernel_spmd(nc, in_maps, list(range(N_CORES)))
    out = np.empty((N_CORES * B, BO, H, T), np.float32)
    for k in range(N_CORES):
        hs = _decode_hs(res.results[k]["hs"])
        out[2 * k : 2 * k + 2] = hs
    return out


def _decode_hs(hs):
    """(B, T//CHUNK, BO, CHUNK*RS) raw ring dump -> (B, BO, H, T) = 2*SH."""
    hs = np.asarray(hs, np.float32).reshape(B, T // CHUNK, BO, CHUNK, RS)
    hs = hs[:, :, :, :, 1 : 1 + H]  # (B, nch, BO, CHUNK, H=row)
    hs = hs.transpose(0, 2, 4, 1, 3).reshape(B, BO, H, T)
    return hs

